# revision 69
# baseline (speedup 1.0000x reference)
"""Sparse-attention (lightning indexer + top-k) Trainium2 kernel, v2.

Sequence-parallel over query rows, 8 cores, zero collectives.
Row pairing for causal balance: core c owns row-blocks c and c+8 (128 rows
each).  Block c attends keys < 1024 (8 chunks), block c+8 keys < 2048 (16
chunks) -- every core does the same (8+16)-chunk attention workload.

v2 changes vs baseline:
  - host pre-transposes x (xT/xloT/xmT/xmloT) -> plain DMA loads, no xbar
    transposes on the critical path at startup.
  - attention computed transposed (att^T chunks via PE, keys on partitions):
    kills the 512 per-head [128,128] DMA transposes that serialized the Sync
    engine for ~640 us.  Only the top-k mask is transposed (8+16 chunks).
  - softmax denominator via ones-vector matmul accumulation; normalization
    through reciprocal_approx_fast on a PE-broadcast of den.
  - v produced as vT (16 LDWEIGHTS instead of 256) then PE-transposed.
  - matmul loops reordered so stationary operands are reused (fewer
    LDWEIGHTS); gate computed transposed (weights stationary).
  - binary search: 16 iters (was 22), Sign(bias=mid, scale=-1) saves the
    negmid op; rt0 counts over 1024 cols only.
"""

import contextlib

import numpy as np
import ml_dtypes

import concourse.bass as bass
import concourse.mybir as mybir
import concourse.tile as tile
from concourse import bacc
from concourse import bass_utils

F32 = mybir.dt.float32
BF16 = mybir.dt.bfloat16
Alu = mybir.AluOpType
Act = mybir.ActivationFunctionType

S = 2048          # sequence length
D = 2048          # model dim
H = 16            # query heads
DH = 128          # head dim
RD = 64           # rope dims
HI = 4            # index heads
DI = 128          # index head dim
TOPK = 512
THETA = 10000.0
NCORES = 8
R = S // NCORES   # rows per core = 256 (two 128-row blocks: c and c+8)
NT = R // 128     # row tiles per core = 2
NC = D // 128     # D chunks = 16
NKC = S // 128    # key chunks = 16
NK = (8, 16)      # key chunks per row tile (block c -> keys<1024, c+8 -> all)
CLAMP = -240.0    # causal clamp value for indexer scores
BS_LO = -60.0     # re-encoded score domain (see baseline notes)
BS_HI = 20.0
BS_ITERS = 19
EPS_STEP = (8.0 / S) * 1.0001   # non-dyadic: avoids sign(0) ties vs mids
ATT_SCALE = float(DH) ** -0.5
IDX_SCALE = float(DI) ** -0.5
WAVE = 3          # attention chunk-wave size (PSUM: 2*3 att + den + ov = 8)


def _hadamard(n):
    h = np.array([[1.0]], dtype=np.float64)
    while h.shape[0] < n:
        h = np.block([[h, h], [h, -h]])
    return h


def _core_rows(c):
    return np.concatenate([np.arange(c * 128, (c + 1) * 128),
                           np.arange((c + 8) * 128, (c + 9) * 128)])


def _host_prep(x, wq, wk, wv, wo, wq_i, wk_i, w_gate):
    xs = x[0]  # [S, D] f32

    Hm = _hadamard(DI) * (DI ** -0.5)
    wqi_f = (wq_i.reshape(D, HI, DI).astype(np.float64) @ Hm).reshape(D, HI * DI)
    wki_f = wk_i.astype(np.float64) @ Hm

    perm = np.concatenate([
        np.arange(0, DH - RD),
        DH - RD + 2 * np.arange(RD // 2),
        DH - RD + 2 * np.arange(RD // 2) + 1,
    ])
    wq_p = wq.reshape(D, H, DH)[:, :, perm].reshape(D, H * DH)
    wk_p = wk[:, perm]

    t = np.arange(S, dtype=np.float64)
    inv = THETA ** (-np.arange(0, RD, 2, dtype=np.float64) / RD)
    ang = t[:, None] * inv[None, :]
    cosT = np.cos(ang).T  # [32, S]
    sinT = np.sin(ang).T

    bf = ml_dtypes.bfloat16

    def split_bf(a):
        a = np.asarray(a, np.float32)
        hi = a.astype(bf)
        lo = (a - hi.astype(np.float32)).astype(bf)
        return hi, lo

    x_hi, x_lo = split_bf(xs)
    wqi_hi, wqi_lo = split_bf(wqi_f.astype(np.float32))
    wki_hi, wki_lo = split_bf(wki_f.astype(np.float32))
    wg_hi, wg_lo = split_bf(w_gate)

    def c_(a):
        return np.ascontiguousarray(a)

    prep = {
        "wq_pk": c_(wq_p.astype(bf).reshape(D, 4, 512).transpose(1, 0, 2)),
        "wk_bf": wk_p.astype(bf),
        "wv_bf": wv.astype(bf),
        "wqi_bf": wqi_hi,
        "wqi_lo": wqi_lo,
        "wki_bf": wki_hi,
        "wki_lo": wki_lo,
        "wg_bf": wg_hi,
        "wg_lo": wg_lo,
        "wo_bf": wo.astype(bf),
        "rk_a": np.concatenate([cosT, sinT], 0).astype(bf),   # [64, S]
        "rk_b": np.concatenate([sinT, cosT], 0).astype(bf),
        "ident": np.eye(128, dtype=np.float32).astype(bf),
        "ident4": np.eye(4, dtype=np.float32),
        "ones_col": np.ones((128, 1), np.float32).astype(bf),
        "ones_row": np.ones((1, 128), np.float32),
    }
    cores = []
    kidx = np.arange(S)
    for c in range(NCORES):
        rows = _core_rows(c)
        cq = np.tile(cosT[:, rows], (1, H))
        sq = np.tile(sinT[:, rows], (1, H))
        causal = np.where(kidx[None, :] <= rows[:, None], 0.0, CLAMP).astype(np.float32)
        cores.append({
            "xmT": c_(x_hi[rows].T),            # [D, R]
            "xmloT": c_(x_lo[rows].T),
            "rq_a": c_(np.concatenate([cq, sq], 0).astype(bf)),
            "rq_b": c_(np.concatenate([sq, cq], 0).astype(bf)),
            "causal": causal.astype(bf),
        })
    return prep, cores


def _rope_block(nc, pool, tT, ta, tb, width):
    """In-place rope on feature-major tile tT (u0 = rows 64:96, u1 = 96:128)."""
    p1 = pool.tile([128, width], BF16, tag="rope_p1")
    p2 = pool.tile([128, width], BF16, tag="rope_p2")
    sh = pool.tile([128, width], BF16, tag="rope_sh")
    nc.vector.tensor_mul(out=p1[64:128, :], in0=tT[64:128, :], in1=ta[64:128, :])
    nc.vector.tensor_mul(out=p2[64:128, :], in0=tT[64:128, :], in1=tb[64:128, :])
    nc.sync.dma_start(out=sh[64:96, :], in_=p1[96:128, :])
    nc.sync.dma_start(out=sh[96:128, :], in_=p2[64:96, :])
    nc.vector.tensor_sub(out=tT[64:96, :], in0=p1[64:96, :], in1=sh[64:96, :])
    nc.vector.tensor_add(out=tT[96:128, :], in0=p2[96:128, :], in1=sh[96:128, :])


def _build(tc, io):
    nc = tc.nc
    xmT_d = io["xmT"]; xmloT_d = io["xmloT"]
    wq_pk = io["wq_pk"]; wk = io["wk_bf"]; wv = io["wv_bf"]
    wqi = io["wqi_bf"]; wqi_lo_d = io["wqi_lo"]
    wki = io["wki_bf"]; wki_lo_d = io["wki_lo"]
    wg = io["wg_bf"]; wg_lo_d = io["wg_lo"]; wo = io["wo_bf"]
    rk_a = io["rk_a"]; rk_b = io["rk_b"]; rq_a = io["rq_a"]; rq_b = io["rq_b"]
    causal = io["causal"]; out = io["out"]

    ctx = contextlib.ExitStack()
    with ctx:
        persist = ctx.enter_context(tc.tile_pool(name="persist", bufs=1))
        kT = persist.tile([128, S], BF16)            # roped k, feature-major
        v_rm = persist.tile([128, NKC, 128], BF16)   # v row-major chunks
        kiT_hi = persist.tile([128, S], BF16)
        kiT_lo = persist.tile([128, S], BF16)
        qT = persist.tile([128, H, R], BF16)         # roped q, feature-major
        qiT_hi = persist.tile([128, HI, R], BF16)
        qiT_lo = persist.tile([128, HI, R], BF16)
        g_sb = persist.tile([128, NT, HI], F32)      # gate (scaled)
        mask = persist.tile([128, NT, S], BF16)      # top-k mask, row-major
        maskT = persist.tile([128, NT, NKC, 128], BF16)
        oT = persist.tile([128, NT, 4, 512], BF16)   # attn out [d, (4h,r)]
        ident = persist.tile([128, 128], BF16)
        ident4 = persist.tile([4, 4], F32)
        ones_c = persist.tile([128, 1], BF16)
        ones_r = persist.tile([1, 128], F32)
        scores0 = persist.tile([128, NK[0] * 128], F32)
        scores1 = persist.tile([128, NK[1] * 128], F32)
        scores_t = (scores0, scores1)

        nc.scalar.dma_start(out=ident[:], in_=io["ident"])
        nc.scalar.dma_start(out=ident4[:], in_=io["ident4"])
        nc.scalar.dma_start(out=ones_c[:], in_=io["ones_col"])
        nc.scalar.dma_start(out=ones_r[:], in_=io["ones_row"])

        # Early small loads (sync ring) + big loads on the scalar HWDGE ring
        # so the first projections (qi/gate, needing only xmT + weights) start
        # within a few us.
        caus_r = causal.rearrange("(t p) s -> p t s", p=128)
        sc_pool = ctx.enter_context(tc.tile_pool(name="sc_misc", bufs=1))
        caus_sb = sc_pool.tile([128, NT, S], BF16)  # 0 / -240: exact in bf16
        nc.scalar.dma_start(out=caus_sb[:, 0, 0:NK[0] * 128],
                            in_=caus_r[:, 0, 0:NK[0] * 128])
        nc.scalar.dma_start(out=caus_sb[:, 1, :], in_=caus_r[:, 1, :])
        eps_t = sc_pool.tile([128, S], F32)
        nc.gpsimd.iota(eps_t[:], pattern=[[1, S]], channel_multiplier=0,
                       allow_small_or_imprecise_dtypes=True)
        nc.vector.tensor_scalar(out=eps_t[:], in0=eps_t[:],
                                scalar1=-EPS_STEP, scalar2=-EPS_STEP,
                                op0=Alu.mult, op1=Alu.add)

        # ================= phase 1a: own-row projections + AllGather =================
        # k / ki / v are shared across cores; each core projects only its own
        # 256 rows and the results are all-gathered (keys are global).
        xt_ctx = contextlib.ExitStack()
        xt_pool = xt_ctx.enter_context(tc.tile_pool(name="xt", bufs=1))
        wpool = xt_ctx.enter_context(tc.tile_pool(name="weights", bufs=1))
        xmT = xt_pool.tile([128, NC, R], BF16)
        xmloT = xt_pool.tile([128, NC, R], BF16)
        vT_g = xt_pool.tile([128, S], BF16)       # gathered vT (until transposed)
        wk_sb = wpool.tile([128, NC, DH], BF16)
        wv_sb = wpool.tile([128, NC, DH], BF16)
        wki_sb = wpool.tile([128, NC, DI], BF16)
        wki_lo_sb = wpool.tile([128, NC, DI], BF16)
        wg_sb = wpool.tile([128, NC, HI], BF16)
        wg_lo_sb = wpool.tile([128, NC, HI], BF16)

        nc.sync.dma_start(out=xmT[:], in_=xmT_d.rearrange("(c p) r -> p c r", p=128))
        nc.sync.dma_start(out=wk_sb[:], in_=wk.rearrange("(c p) d -> p c d", p=128))
        nc.sync.dma_start(out=wv_sb[:], in_=wv.rearrange("(c p) d -> p c d", p=128))
        nc.sync.dma_start(out=wki_sb[:], in_=wki.rearrange("(c p) d -> p c d", p=128))
        nc.sync.dma_start(out=wki_lo_sb[:],
                          in_=wki_lo_d.rearrange("(c p) d -> p c d", p=128))
        nc.scalar.dma_start(out=wg_sb[:], in_=wg.rearrange("(c p) d -> p c d", p=128))
        nc.scalar.dma_start(out=wg_lo_sb[:],
                            in_=wg_lo_d.rearrange("(c p) d -> p c d", p=128))
        nc.scalar.dma_start(out=xmloT[:],
                            in_=xmloT_d.rearrange("(c p) r -> p c r", p=128))

        wqbig_pool = xt_ctx.enter_context(tc.tile_pool(name="wq_big", bufs=2))
        with tc.tile_pool(name="wqi_stream", bufs=2) as wqi_pool, \
             tc.tile_pool(name="stage", bufs=1) as stage_pool, \
             tc.tile_pool(name="cc_dram", bufs=1, space="DRAM") as dram_pool, \
             tc.tile_pool(name="p1a_psum", bufs=1, space="PSUM") as ppool, \
             tc.tile_pool(name="gt_sb", bufs=1) as gt_pool:

            # wq loaded as four 2 MB pieces, issued up front on the scalar
            # ring so the q matmuls (which precede the scores in the PE
            # stream) are never weight-starved.
            wq_gs = []
            for grp in range(4):
                wq_g = wqbig_pool.tile([128, NC, 512], BF16, tag="wq_g",
                                       name=f"wq_g{grp}")
                nc.scalar.dma_start(
                    out=wq_g[:],
                    in_=wq_pk[grp].rearrange("(c p) d -> p c d", p=128))
                wq_gs.append(wq_g)

            # --- own-row k/ki/v projections: [128, R] each, 3-term for ki ---
            ps_own = ppool.tile([128, 4, 512], F32, tag="own", name="ps_own")
            for c in range(NC):
                nc.tensor.matmul(ps_own[:, 0, 0:R], lhsT=wk_sb[:, c, :],
                                 rhs=xmT[:, c, :],
                                 start=(c == 0), stop=(c == NC - 1))
                nc.tensor.matmul(ps_own[:, 1, 0:R], lhsT=wv_sb[:, c, :],
                                 rhs=xmT[:, c, :],
                                 start=(c == 0), stop=(c == NC - 1))
                nc.tensor.matmul(ps_own[:, 2, 0:R], lhsT=wki_sb[:, c, :],
                                 rhs=xmT[:, c, :],
                                 start=(c == 0), stop=False)
                nc.tensor.matmul(ps_own[:, 2, 0:R], lhsT=wki_sb[:, c, :],
                                 rhs=xmloT[:, c, :],
                                 start=False, stop=False)
                nc.tensor.matmul(ps_own[:, 2, 0:R], lhsT=wki_lo_sb[:, c, :],
                                 rhs=xmT[:, c, :],
                                 start=False, stop=(c == NC - 1))
            # stage: [128, (ki_hi, ki_lo, kT_own, vT_own) x 256] bf16
            stage = stage_pool.tile([128, 4, R], BF16)
            nc.vector.tensor_copy(stage[:, 0, :], ps_own[:, 2, 0:R])
            nc.vector.tensor_sub(out=stage[:, 1, :], in0=ps_own[:, 2, 0:R],
                                 in1=stage[:, 0, :])
            nc.vector.tensor_copy(stage[:, 2, :], ps_own[:, 0, 0:R])
            nc.vector.tensor_copy(stage[:, 3, :], ps_own[:, 1, 0:R])
            in_bounce = dram_pool.tile([128, 4 * R], BF16)
            out_bounce = dram_pool.tile([NCORES, 128, 4 * R], BF16)
            nc.gpsimd.dma_start(in_bounce[:],
                                stage[:].rearrange("p t r -> p (t r)"))
            nc.gpsimd.collective_compute(
                "AllGather", mybir.AluOpType.bypass,
                replica_groups=[list(range(NCORES))],
                ins=[in_bounce.opt()],
                outs=[out_bounce.opt()],
            )
            # gathered columns come back block-interleaved: core c holds
            # blocks (c, c+8) -> column order (b, c, s) covers 0..2047
            for t, (dst, eng) in enumerate([(kiT_hi, nc.sync), (kiT_lo, nc.sync),
                                            (kT, nc.scalar), (vT_g, nc.scalar)]):
                for b in range(2):
                    eng.dma_start(
                        out=dst[:, bass.ts(b, 1024)]
                        .rearrange("p (c s) -> p c s", c=NCORES),
                        in_=out_bounce[:, :, bass.ds(t * 256 + b * 128, 128)]
                        .rearrange("c p s -> p c s"))

            # --- qi: 3-term split precision, weights in 4-chunk pieces ---
            psqi = ppool.tile([128, HI, 512], F32, tag="proj_big", name="psqi")
            for c4 in range(NC // 4):
                wqi_c = wqi_pool.tile([128, 4, HI * DI], BF16, tag="wqi_c")
                wqi_lo_c = wqi_pool.tile([128, 4, HI * DI], BF16, tag="wqi_lo_c")
                nc.sync.dma_start(
                    out=wqi_c[:],
                    in_=wqi[bass.ts(c4, 512), :].rearrange("(c p) d -> p c d", p=128))
                nc.sync.dma_start(
                    out=wqi_lo_c[:],
                    in_=wqi_lo_d[bass.ts(c4, 512), :].rearrange("(c p) d -> p c d", p=128))
                for cc in range(4):
                    c = c4 * 4 + cc
                    for hi in range(HI):
                        dsl = bass.ts(hi, DI)
                        nc.tensor.matmul(psqi[:, hi, 0:R],
                                         lhsT=wqi_c[:, cc, dsl],
                                         rhs=xmT[:, c, :],
                                         start=(c == 0), stop=False)
                        nc.tensor.matmul(psqi[:, hi, 0:R],
                                         lhsT=wqi_c[:, cc, dsl],
                                         rhs=xmloT[:, c, :],
                                         start=False, stop=False)
                        nc.tensor.matmul(psqi[:, hi, 0:R],
                                         lhsT=wqi_lo_c[:, cc, dsl],
                                         rhs=xmT[:, c, :],
                                         start=False, stop=(c == NC - 1))
            nc.scalar.copy(qiT_hi[:], psqi[:, :, 0:R])
            nc.vector.tensor_sub(out=qiT_lo[:], in0=psqi[:, :, 0:R], in1=qiT_hi[:])

            # --- gate, transposed: gT[hi, r] with weights stationary ---
            psg = ppool.tile([4, 512], F32, tag="own", name="psg")
            for c in range(NC):
                nc.tensor.matmul(psg[0:4, 0:R], lhsT=wg_sb[:, c, :],
                                 rhs=xmT[:, c, :], start=(c == 0), stop=False)
                nc.tensor.matmul(psg[0:4, 0:R], lhsT=wg_sb[:, c, :],
                                 rhs=xmloT[:, c, :], start=False, stop=False)
                nc.tensor.matmul(psg[0:4, 0:R], lhsT=wg_lo_sb[:, c, :],
                                 rhs=xmT[:, c, :], start=False,
                                 stop=(c == NC - 1))
            gT_sb = gt_pool.tile([4, R], F32)
            nc.vector.tensor_copy(gT_sb[:], psg[0:4, 0:R])
            for rt in range(NT):
                psgt = ppool.tile([128, HI], F32, tag="own", name=f"psgt{rt}")
                nc.tensor.matmul(psgt[:, 0:4], lhsT=gT_sb[:, bass.ts(rt, 128)],
                                 rhs=ident4[:], is_transpose=True)
                nc.vector.tensor_scalar(out=g_sb[:, rt, :], in0=psgt[:, 0:4],
                                        scalar1=IDX_SCALE, scalar2=None,
                                        op0=Alu.mult)

        # ====== phase 2 + phase 1b (one scope: search overlaps projections) ======
        # Scores psum is built in 1024-wide halves (2 banks, bufs=2 -> 4 banks)
        # so it coexists with the projection psum (4 banks) -- no cross-pool
        # release dependency stalls the PE during the serial binary search.
        with tc.tile_pool(name="sc_tmp", bufs=2) as tmp_pool, \
             tc.tile_pool(name="idx_psum", bufs=2, space="PSUM") as idx_psum, \
             tc.tile_pool(name="bs", bufs=1) as bs_pool, \
             tc.tile_pool(name="ztmp_pool", bufs=1) as ztmp_pool, \
             tc.tile_pool(name="rope", bufs=1) as rope_pool, \
             tc.tile_pool(name="p1b_psum", bufs=1, space="PSUM") as ppool:

            # --- q projection first: weights pre-loaded in big pieces, so
            # the PE stream has work while the kiT AllGather is in flight ---
            for grp in range(4):
                psq = ppool.tile([128, 4, 512], F32, tag="proj_big",
                                 name=f"psq{grp}")
                for c in range(NC):
                    for h4 in range(4):
                        nc.tensor.matmul(psq[:, h4, 0:R],
                                         lhsT=wq_gs[grp][:, c, bass.ts(h4, DH)],
                                         rhs=xmT[:, c, :],
                                         start=(c == 0), stop=(c == NC - 1))
                nc.scalar.copy(qT[:, bass.ds(grp * 4, 4), :], psq[:, :, 0:R])

            for rt in range(NT):
                W = NK[rt] * 128
                scores = scores_t[rt]
                for half in range(W // 1024):
                    hsl = bass.ds(half * 1024, 1024)
                    for hi in range(HI):
                        psl = idx_psum.tile([128, 1024], F32, tag="logits")
                        rsl = bass.ts(rt, 128)
                        for nq in range(2):
                            sl = bass.ds(half * 1024 + nq * 512, 512)
                            nc.tensor.matmul(psl[:, bass.ts(nq, 512)],
                                             lhsT=qiT_hi[:, hi, rsl],
                                             rhs=kiT_hi[:, sl],
                                             start=True, stop=False)
                            nc.tensor.matmul(psl[:, bass.ts(nq, 512)],
                                             lhsT=qiT_hi[:, hi, rsl],
                                             rhs=kiT_lo[:, sl],
                                             start=False, stop=False)
                            nc.tensor.matmul(psl[:, bass.ts(nq, 512)],
                                             lhsT=qiT_lo[:, hi, rsl],
                                             rhs=kiT_hi[:, sl],
                                             start=False, stop=True)
                        tmp = tmp_pool.tile([128, 1024], F32, tag="relu_t")
                        nc.vector.tensor_scalar(out=tmp[:], in0=psl[:],
                                                scalar1=0.0,
                                                scalar2=g_sb[:, rt, hi:hi + 1],
                                                op0=Alu.max, op1=Alu.mult)
                        if hi == 0:
                            nc.vector.tensor_add(out=scores[:, hsl], in0=tmp[:],
                                                 in1=caus_sb[:, rt, hsl])
                        else:
                            nc.vector.tensor_add(out=scores[:, hsl],
                                                 in0=scores[:, hsl], in1=tmp[:])
                # rank re-encode: negatives -10; exact zeros -> eps cluster
                ztmp = ztmp_pool.tile([128, S], F32, tag="ztmp")
                nc.vector.tensor_scalar(out=ztmp[:, 0:W], in0=scores[:],
                                        scalar1=0.0, scalar2=-10.0,
                                        op0=Alu.is_lt, op1=Alu.mult)
                nc.vector.tensor_add(out=scores[:], in0=scores[:],
                                     in1=ztmp[:, 0:W])
                nc.vector.scalar_tensor_tensor(out=ztmp[:, 0:W], in0=scores[:],
                                               scalar=0.0, in1=eps_t[:, 0:W],
                                               op0=Alu.is_equal, op1=Alu.mult)
                nc.vector.tensor_add(out=scores[:], in0=scores[:],
                                     in1=ztmp[:, 0:W])

            # --- v_rm: PE-transpose the gathered vT ---
            for kc in range(NKC):
                pst = ppool.tile([128, 128], BF16, tag="proj_big")
                nc.tensor.transpose(pst[:], vT_g[:, bass.ts(kc, 128)], ident[:])
                nc.vector.tensor_copy(v_rm[:, kc, :], pst[:])

            # --- rope (emitted before the searches so its DVE ops are not
            # queued behind the serial search on the vector engine) ---
            ta = rope_pool.tile([128, S], BF16, tag="ta")
            tb = rope_pool.tile([128, S], BF16, tag="tb")
            nc.sync.dma_start(out=ta[64:128, :], in_=rk_a)
            nc.sync.dma_start(out=tb[64:128, :], in_=rk_b)
            _rope_block(nc, rope_pool, kT, ta, tb, S)
            qT_flat = qT[:].rearrange("p h r -> p (h r)")
            for half in range(2):
                ta2 = rope_pool.tile([128, S], BF16, tag="ta")
                tb2 = rope_pool.tile([128, S], BF16, tag="tb")
                nc.sync.dma_start(out=ta2[64:128, :], in_=rq_a[:, bass.ts(half, S)])
                nc.sync.dma_start(out=tb2[64:128, :], in_=rq_b[:, bass.ts(half, S)])
                _rope_block(nc, rope_pool, qT_flat[:, bass.ts(half, S)], ta2, tb2, S)

            # --- binary searches: rt0 fully first so its mask unblocks
            # attention(rt0) while rt1's search still runs ---
            for rt in range(NT):
                W = NK[rt] * 128
                THR = float(W - 2 * TOPK)
                lo = bs_pool.tile([128, 1], F32, tag=f"lo{rt}", name=f"lo{rt}")
                mid = bs_pool.tile([128, 1], F32, tag=f"mid{rt}", name=f"mid{rt}")
                cnt = bs_pool.tile([128, 1], F32, tag=f"cnt{rt}", name=f"cnt{rt}")
                bge = bs_pool.tile([128, 1], mybir.dt.uint32, tag=f"bge{rt}",
                                   name=f"bge{rt}")
                dlt = bs_pool.tile([128, 1], F32, tag=f"dlt{rt}", name=f"dlt{rt}")
                nc.vector.memset(lo[:], BS_LO)
                nc.vector.memset(mid[:], 0.5 * (BS_LO + BS_HI))
                step = 0.25 * (BS_HI - BS_LO)
                for it in range(BS_ITERS):
                    sgn = ztmp_pool.tile([128, S], BF16, tag="ztmp")
                    nc.scalar.activation(sgn[:, 0:W], scores_t[rt][:], Act.Sign,
                                         bias=mid[:], scale=-1.0,
                                         accum_out=cnt[:])
                    nc.vector.tensor_scalar(out=bge[:], in0=cnt[:],
                                            scalar1=THR, scalar2=None,
                                            op0=Alu.is_le)
                    nc.vector.copy_predicated(lo[:], bge[:], mid[:])
                    if it < BS_ITERS - 1:
                        # mid += step*(2*bge - 1); step halves each iter
                        nc.vector.tensor_scalar(out=dlt[:], in0=bge[:],
                                                scalar1=2.0 * step,
                                                scalar2=-step,
                                                op0=Alu.mult, op1=Alu.add)
                        nc.vector.tensor_add(out=mid[:], in0=mid[:], in1=dlt[:])
                        step *= 0.5
                nc.vector.tensor_scalar(out=mask[:, rt, 0:W], in0=scores_t[rt][:],
                                        scalar1=lo[:], scalar2=None,
                                        op0=Alu.is_ge)
                # transpose mask chunks (alternate the two HWDGE rings)
                for kc in range(NK[rt]):
                    eng = nc.sync if kc % 2 == 0 else nc.scalar
                    eng.dma_start(out=maskT[:, rt, kc, :],
                                  in_=mask[:, rt, bass.ts(kc, 128)],
                                  transpose=True)

        xt_ctx.close()   # frees xmT etc. before the wo preload

        # wo preload: 8 MB lands during phase 3 (scalar ring)
        wo_pool = ctx.enter_context(tc.tile_pool(name="wo_sb", bufs=1))
        wo_sb = wo_pool.tile([128, H, D], BF16)
        nc.scalar.dma_start(out=wo_sb[:], in_=wo.rearrange("(h p) e -> p h e", p=128))

        # ==================== phase 3: attention (transposed) ====================
        with tc.tile_pool(name="p_pool", bufs=7) as p_pool, \
             tc.tile_pool(name="rd_pool", bufs=2) as rd_pool, \
             tc.tile_pool(name="att_psum", bufs=2, space="PSUM") as att_psum, \
             tc.tile_pool(name="den_psum", bufs=1, space="PSUM") as den_psum, \
             tc.tile_pool(name="ov_psum", bufs=1, space="PSUM") as ov_psum:

            for rt in range(NT):
                NK_ = NK[rt]
                for g in range(4):
                    qTg = qT[:, bass.ds(4 * g, 4), bass.ts(rt, 128)]
                    den = den_psum.tile([1, 512], F32, tag="den")
                    ov = ov_psum.tile([128, 512], F32, tag="ov")
                    kc0 = 0
                    while kc0 < NK_:
                        w = min(WAVE, NK_ - kc0)
                        att = att_psum.tile([128, WAVE, 512], F32, tag="att")
                        for j in range(w):
                            kc = kc0 + j
                            nc.tensor.matmul(att[:, j, :],
                                             lhsT=kT[:, bass.ts(kc, 128)],
                                             rhs=qTg,
                                             start=True, stop=True)
                        p_sb = p_pool.tile([128, WAVE, 512], BF16, tag="p")
                        nc.scalar.activation(p_sb[:, 0:w, :], att[:, 0:w, :],
                                             Act.Exp, scale=ATT_SCALE)
                        pm = p_pool.tile([128, WAVE, 512], BF16, tag="pm")
                        mbc = maskT[:, rt, kc0:kc0 + w, :] \
                            .unsqueeze(2).broadcast_to([128, w, 4, 128])
                        nc.vector.tensor_mul(
                            out=pm[:, 0:w, :].rearrange("p c (h r) -> p c h r", h=4),
                            in0=p_sb[:, 0:w, :].rearrange("p c (h r) -> p c h r", h=4),
                            in1=mbc)
                        for j in range(w):
                            kc = kc0 + j
                            nc.tensor.matmul(den[0:1, :], lhsT=ones_c[:, :],
                                             rhs=pm[:, j, :],
                                             start=(kc == 0), stop=(kc == NK_ - 1))
                        for j in range(w):
                            kc = kc0 + j
                            nc.tensor.matmul(ov[:], lhsT=v_rm[:, kc, :],
                                             rhs=pm[:, j, :],
                                             start=(kc == 0), stop=(kc == NK_ - 1))
                        kc0 += w
                    den_sb = rd_pool.tile([1, 512], F32, tag="den_sb")
                    nc.vector.tensor_copy(den_sb[:], den[0:1, :])
                    den_bc = att_psum.tile([128, WAVE, 512], F32, tag="att")
                    nc.tensor.matmul(den_bc[:, 0, :], lhsT=ones_r[:, :],
                                     rhs=den_sb[:], start=True, stop=True)
                    rd_sb = rd_pool.tile([128, 512], F32, tag="rd_sb")
                    nc.vector.reciprocal_approx_fast(out=rd_sb[:],
                                                     in_=den_bc[:, 0, :])
                    nc.vector.tensor_mul(out=oT[:, rt, g, :], in0=ov[:],
                                         in1=rd_sb[:])

        # ================= phase 4: output projection =================
        with tc.tile_pool(name="o_sb", bufs=2) as o_sb_pool, \
             tc.tile_pool(name="out_psum", bufs=1, space="PSUM") as out_psum:

            accs = [out_psum.tile([128, 512], F32, tag=f"acc{i}", name=f"acc{i}")
                    for i in range(2 * 4)]
            for h in range(H):
                g, h4 = h // 4, h % 4
                for rt in range(NT):
                    for ec in range(4):
                        nc.tensor.matmul(accs[rt * 4 + ec][:],
                                         lhsT=oT[:, rt, g, bass.ts(h4, 128)],
                                         rhs=wo_sb[:, h, bass.ts(ec, 512)],
                                         start=(h == 0), stop=(h == H - 1))
            for rt in range(NT):
                for ec in range(4):
                    o_sb = o_sb_pool.tile([128, 512], F32, tag="o_out")
                    nc.vector.tensor_copy(o_sb[:], accs[rt * 4 + ec][:])
                    nc.sync.dma_start(
                        out=out[bass.ts(rt, 128), bass.ts(ec, 512)],
                        in_=o_sb[:])


_CACHED = {}


def _get_nc():
    if "nc" in _CACHED:
        return _CACHED["nc"], _CACHED["names"]
    nc = bacc.Bacc("TRN2", target_bir_lowering=False, debug=False,
                   enable_asserts=False, num_devices=NCORES)
    specs = {
        "xmT": ((D, R), BF16),
        "xmloT": ((D, R), BF16),
        "wq_pk": ((4, D, 512), BF16),
        "wk_bf": ((D, DH), BF16),
        "wv_bf": ((D, DH), BF16),
        "wqi_bf": ((D, HI * DI), BF16),
        "wqi_lo": ((D, HI * DI), BF16),
        "wki_bf": ((D, DI), BF16),
        "wki_lo": ((D, DI), BF16),
        "wg_bf": ((D, HI), BF16),
        "wg_lo": ((D, HI), BF16),
        "wo_bf": ((H * DH, D), BF16),
        "rk_a": ((64, S), BF16),
        "rk_b": ((64, S), BF16),
        "rq_a": ((64, H * R), BF16),
        "rq_b": ((64, H * R), BF16),
        "causal": ((R, S), BF16),
        "ident": ((128, 128), BF16),
        "ident4": ((4, 4), F32),
        "ones_col": ((128, 1), BF16),
        "ones_row": ((1, 128), F32),
    }
    io = {}
    for name, (shape, dt) in specs.items():
        io[name] = nc.dram_tensor(name, shape, dt, kind="ExternalInput").ap()
    io["out"] = nc.dram_tensor("out", (R, D), F32, kind="ExternalOutput").ap()
    with tile.TileContext(nc) as tc:
        _build(tc, io)
    nc.compile()
    _CACHED["nc"] = nc
    _CACHED["names"] = list(specs.keys())
    return nc, _CACHED["names"]


TRACE = False
LAST_RESULT = None
TRACE_DIR = None


def kernel(x, wq, wk, wv, wo, wq_i, wk_i, w_gate):
    global LAST_RESULT
    prep, cores = _host_prep(
        np.asarray(x, np.float32), np.asarray(wq, np.float32),
        np.asarray(wk, np.float32), np.asarray(wv, np.float32),
        np.asarray(wo, np.float32), np.asarray(wq_i, np.float32),
        np.asarray(wk_i, np.float32), np.asarray(w_gate, np.float32))
    nc, names = _get_nc()
    in_maps = []
    for c in range(NCORES):
        m = {}
        for n in names:
            m[n] = np.ascontiguousarray(cores[c][n] if n in cores[c] else prep[n])
        in_maps.append(m)
    kwargs = {}
    if TRACE:
        kwargs = dict(trace=True, tmpdir=TRACE_DIR)
    res = bass_utils.run_bass_kernel_spmd(nc, in_maps,
                                          core_ids=list(range(NCORES)), **kwargs)
    LAST_RESULT = res
    full = np.empty((S, D), np.float32)
    for c in range(NCORES):
        o = res.results[c]["out"]
        full[c * 128:(c + 1) * 128] = o[0:128]
        full[(c + 8) * 128:(c + 9) * 128] = o[128:256]
    return full[None]
